# revision 1
# baseline (speedup 1.0000x reference)
"""GAT (4-layer, 8-head) Trainium2 kernel, 8-core SPMD.

Strategy (per sharding hint): nodes partitioned into 8 contiguous shards
(4096 nodes/core); each core owns the edges whose dst falls in its shard
(segment softmax + scatter stay local). Per-layer, each core computes the
dense ft = h @ W for its shard, then an AllGather replicates ft (+ the
attention logit components el/er) so every core can gather remote src rows
with dma_gather. Edge aggregation (segment softmax + weighted scatter-add)
is done as PE matmuls against host-precomputed one-hot dst matrices, with
the per-edge exp() folded into the moving operand. Weights are replicated.

Softmax: the reference subtracts a per-segment max; mathematically alpha is
invariant to any constant shift, and the logit range here is [-7, 7], so we
exp() directly (f32 safe) and normalize by the segment sum at the node side.
"""

import functools

import numpy as np

import concourse.bacc as bacc
import concourse.bass as bass
import concourse.mybir as mybir
import concourse.tile as tile
from concourse.bass_utils import run_bass_kernel_spmd

# ---- problem constants (hardcoded per contract) ----
N, E, G = 32768, 262144, 64
NCORES = 8
SH = N // NCORES          # 4096 nodes per core
NB = SH // 128            # 32 dst blocks per core
NCH = NB                  # node chunks per core (same 128-partition blocks)
F0, F = 128, 512
H, D = 8, 64
H3 = 6
NEG_SLOPE = 0.2
EPS = 1e-30

f32 = mybir.dt.float32
f32r = mybir.dt.float32r
bf16 = mybir.dt.bfloat16
i16 = mybir.dt.int16

# dtype knobs: storage dtype of the gathered ft / one-hot S / edge moving
# operand (f32 or bf16), matmul compute dtype for f32 tiles (f32r = full-rate)
FT_DT = f32
MM_DT = f32r

TRACE = False
TRACE_KW = {}
LAST = {}

AF = mybir.ActivationFunctionType
ALU = mybir.AluOpType
AX = mybir.AxisListType


def _wrap_idx(v):
    """int16 gather-index layout: element i at [i%16, i//16], replicated to
    128 partitions."""
    L = len(v)
    w = np.zeros((16, L // 16), np.int16)
    w[np.arange(L) % 16, np.arange(L) // 16] = v.astype(np.int16)
    return np.tile(w, (8, 1))


def preprocess(inputs):
    src = np.asarray(inputs["src"]).astype(np.int64)
    dst = np.asarray(inputs["dst"]).astype(np.int64)
    graph_id = np.asarray(inputs["graph_id"]).astype(np.int64)
    feat = np.asarray(inputs["feat"], dtype=np.float32)

    per_core_edges = []
    KB = 0
    for c in range(NCORES):
        m = (dst >= c * SH) & (dst < (c + 1) * SH)
        es, ed = src[m], dst[m]
        o = np.argsort(ed, kind="stable")
        es, ed = es[o], ed[o]
        dl = ed - c * SH
        blk = dl >> 7
        counts = np.bincount(blk, minlength=NB)
        KB = max(KB, int(np.ceil(counts.max() / 128)))
        per_core_edges.append((es, ed, dl, blk, counts))
    EB = KB * 128

    ft_np = np.float32 if FT_DT == f32 else np.dtype("bfloat16")
    try:
        np.zeros(1, ft_np)
    except TypeError:
        import ml_dtypes
        ft_np = ml_dtypes.bfloat16

    # shared weight-derived arrays
    def Amat(al):  # [1,H,D] -> [H*D, H]
        al = np.asarray(al, np.float64)[0]
        hh, dd = al.shape
        A = np.zeros((hh * dd, hh), np.float64)
        for h in range(hh):
            A[h * dd:(h + 1) * dd, h] = al[h]
        return A

    W0 = np.asarray(inputs["W0"], np.float64)
    W1 = np.asarray(inputs["W1"], np.float64)
    W2 = np.asarray(inputs["W2"], np.float64)
    W3 = np.asarray(inputs["W3"], np.float64)
    resW3 = np.asarray(inputs["resW3"], np.float64)
    al3 = np.asarray(inputs["al3"], np.float64)[0, :, 0]
    ar3 = np.asarray(inputs["ar3"], np.float64)[0, :, 0]

    WA = {}
    for l, W in ((0, W0), (1, W1), (2, W2)):
        Aal = Amat(inputs[f"al{l}"])
        Aar = Amat(inputs[f"ar{l}"])
        WA[l] = np.concatenate([W @ Aal, W @ Aar], axis=1).astype(np.float32)
    W3c = np.concatenate(
        [W3, W3 * al3[None, :], W3 * ar3[None, :], resW3], axis=1
    ).astype(np.float32)  # [512, 24]
    b3row = np.zeros((1, 24), np.float32)
    b3row[0, 18:24] = np.asarray(inputs["b3"], np.float32)

    bias_bc = np.tile(
        np.concatenate([np.asarray(inputs[f"b{l}"], np.float32)
                        for l in range(3)])[None, :], (128, 1))  # [128, 3*512]
    lin_bc = np.zeros((128, H3 + 1), np.float32)
    lin_bc[:, 0:H3] = np.asarray(inputs["linW"], np.float32)[:, 0][None, :]
    lin_bc[:, H3] = float(np.asarray(inputs["linb"], np.float32)[0])
    ident = np.eye(128, dtype=np.float32)

    shared = {
        "W0": W0.astype(np.float32),
        "W1": W1.astype(np.float32),
        "W2": W2.astype(np.float32),
        "W3c": W3c,
        "WA0": WA[0], "WA1": WA[1], "WA2": WA[2],
        "b3row": b3row,
        "bias_bc": bias_bc,
        "lin_bc": lin_bc,
        "identity": ident,
        "ones1": np.ones((1, 128), np.float32),
    }

    in_maps = []
    eye64 = np.eye(G, dtype=np.float32)
    for c in range(NCORES):
        es, ed, dl, blk, counts = per_core_edges[c]
        offs = np.concatenate([[0], np.cumsum(counts)])
        idxX = np.zeros((128, NB * EB // 16), np.int16)
        idxE = np.zeros((128, NB * EB // 16), np.int16)
        Sarr = np.zeros((NB, 128, EB), np.float32)
        for b in range(NB):
            s_b = es[offs[b]:offs[b + 1]]
            d_b = ed[offs[b]:offs[b + 1]]
            dloc = dl[offs[b]:offs[b + 1]] - b * 128
            npad = EB - len(s_b)
            s_pad = np.concatenate([s_b, np.zeros(npad, np.int64)])
            dg_pad = np.concatenate([d_b, np.zeros(npad, np.int64)])
            idxX[:, b * EB // 16:(b + 1) * EB // 16] = _wrap_idx(s_pad)
            idxE[:, b * EB // 16:(b + 1) * EB // 16] = _wrap_idx(dg_pad)
            j = np.arange(len(dloc))
            S3 = Sarr[b].reshape(128, KB, 128)
            S3[j % 128, j // 128, dloc] = 1.0
        gid = graph_id[c * SH:(c + 1) * SH]
        Gh = eye64[gid].reshape(NB, 128, G)
        im = dict(shared)
        im["feat_sh"] = feat[c * SH:(c + 1) * SH]
        im["idxX"] = idxX
        im["idxE"] = idxE
        im["Sarr"] = Sarr.astype(ft_np)
        im["Gh"] = Gh
        in_maps.append(im)
    return in_maps, KB


@functools.lru_cache(maxsize=4)
def build_program(KB, ft_dt_name, debug_dump=False, phases=8, nb_lim=NB,
                  l3_lim=3):
    FT = {"float32": f32r, "bfloat16": bf16}[ft_dt_name]
    EB = KB * 128
    nc = bacc.Bacc("TRN2", target_bir_lowering=False, debug=False)

    # ---- I/O ----
    feat_sh = nc.dram_tensor("feat_sh", [SH, F0], f32, kind="ExternalInput")
    Wt = {
        0: nc.dram_tensor("W0", [F0, F], f32r, kind="ExternalInput"),
        1: nc.dram_tensor("W1", [F, F], f32r, kind="ExternalInput"),
        2: nc.dram_tensor("W2", [F, F], f32r, kind="ExternalInput"),
        3: nc.dram_tensor("W3c", [F, 24], f32r, kind="ExternalInput"),
    }
    WAt = {l: nc.dram_tensor(f"WA{l}", [F0 if l == 0 else F, 16], f32r,
                             kind="ExternalInput") for l in range(3)}
    b3row = nc.dram_tensor("b3row", [1, 24], f32r, kind="ExternalInput")
    bias_bc = nc.dram_tensor("bias_bc", [128, 3 * F], f32, kind="ExternalInput")
    lin_bc = nc.dram_tensor("lin_bc", [128, H3 + 1], f32, kind="ExternalInput")
    identity = nc.dram_tensor("identity", [128, 128], f32, kind="ExternalInput")
    ones1 = nc.dram_tensor("ones1", [1, 128], f32r, kind="ExternalInput")
    idxX = nc.dram_tensor("idxX", [128, NB * EB // 16], i16, kind="ExternalInput")
    idxE = nc.dram_tensor("idxE", [128, NB * EB // 16], i16, kind="ExternalInput")
    Sarr = nc.dram_tensor("Sarr", [NB, 128, EB], FT, kind="ExternalInput")
    Gh = nc.dram_tensor("Gh", [NB, 128, G], f32r, kind="ExternalInput")
    out = nc.dram_tensor("out", [G, 1], f32, kind="ExternalOutput")
    dbg = {}
    if debug_dump:
        for nm, shp in (("dbg_h1", [SH, F]), ("dbg_h2", [SH, F]),
                        ("dbg_h3", [SH, F]), ("dbg_ft0", [SH, F]),
                        ("dbg_eler0", [SH, 64]), ("dbg_hfin", [SH, H3]),
                        ("dbg_pol", [G, H3]), ("dbg_pol2", [G, H3])):
            dbg[nm] = nc.dram_tensor(nm, shp, f32, kind="ExternalOutput")

    rg = [list(range(NCORES))]

    with tile.TileContext(nc) as tc:
        with (
            tc.tile_pool(name="const", bufs=1) as constp,
            tc.tile_pool(name="wpool", bufs=2) as wpool,
            tc.tile_pool(name="work", bufs=3) as work,
            tc.tile_pool(name="edge", bufs=2) as edge,
            tc.tile_pool(name="psA", bufs=2, space="PSUM") as psA,
            tc.tile_pool(name="psB", bufs=2, space="PSUM") as psB,
            tc.tile_pool(name="psC", bufs=2, space="PSUM") as psC,
            tc.tile_pool(name="psP", bufs=1, space="PSUM") as psP,
            tc.tile_pool(name="dram", bufs=1, space="DRAM") as dram,
        ):
            # ---- resident constants ----
            ident_sb = constp.tile([128, 128], f32)
            nc.sync.dma_start(ident_sb[:], identity[:])
            ones_sb = constp.tile([1, 128], f32r)
            nc.sync.dma_start(ones_sb[:], ones1[:])
            b3r_sb = constp.tile([1, 24], f32r)
            nc.sync.dma_start(b3r_sb[:], b3row[:])
            lin_sb = constp.tile([128, H3 + 1], f32)
            nc.sync.dma_start(lin_sb[:], lin_bc[:])
            bias_sb = constp.tile([128, 3 * F], f32)
            nc.sync.dma_start(bias_sb[:], bias_bc[:])
            res3_sb = constp.tile([128, NCH * H3], f32)

            # ---- internal DRAM arrays ----
            FR = F + 64  # packed row: ft(512) | el(8) er(8) pad(48)
            h_in = {l: dram.tile([SH, F], f32, name=f"h{l}") for l in (1, 2, 3)}
            ftag = {l: dram.tile([SH, FR], FT, name=f"ftag{l}") for l in range(3)}
            ftg = {l: dram.tile([N, FR], FT, name=f"ftg{l}") for l in range(3)}
            ft3ag = dram.tile([SH, 64], f32, name="ft3ag")
            ft3g = dram.tile([N, 64], f32, name="ft3g")
            ar_in = dram.tile([G, H3], f32, name="arin")
            ar_out = dram.tile([G, H3], f32, name="arout")

            # ================= dense phase =================
            def dense_phase(l):
                K = F0 if l == 0 else F
                KBl = K // 128
                # layer weights -> SBUF
                w_sb = wpool.tile([128, KBl, F if l < 3 else 24], f32r, tag="W")
                nc.sync.dma_start(
                    w_sb[:], Wt[l][:].rearrange("(kb p) f -> p kb f", p=128))
                if l < 3:
                    wa_sb = wpool.tile([128, KBl, 16], f32r, tag="WA")
                    nc.sync.dma_start(
                        wa_sb[:], WAt[l][:].rearrange("(kb p) f -> p kb f", p=128))
                h_src = feat_sh if l == 0 else h_in[l]
                for j in range(NCH):
                    hl = work.tile([128, K], f32, tag="hload")
                    nc.sync.dma_start(hl[:], h_src[j * 128:(j + 1) * 128, :])
                    pT = psA.tile([128, K], f32, tag="A")
                    for kb in range(KBl):
                        nc.tensor.transpose(
                            pT[:, kb * 128:(kb + 1) * 128],
                            hl[:, kb * 128:(kb + 1) * 128], ident_sb[:])
                    hT = work.tile([128, K], f32r, tag="hT")
                    nc.scalar.copy(hT[:], pT[:])
                    FW = F if l < 3 else 24
                    pft = psB.tile([128, FW], f32, tag="B")
                    for kb in range(KBl):
                        nc.tensor.matmul(
                            pft[:], hT[:, kb * 128:(kb + 1) * 128],
                            w_sb[:, kb, :],
                            start=(kb == 0), stop=(kb == KBl - 1 and l < 3))
                    if l == 3:
                        nc.tensor.matmul(pft[:], ones_sb[:], b3r_sb[:],
                                         start=False, stop=True)
                    if l < 3:
                        pel = psC.tile([128, 16], f32, tag="C")
                        for kb in range(KBl):
                            nc.tensor.matmul(
                                pel[:], hT[:, kb * 128:(kb + 1) * 128],
                                wa_sb[:, kb, :],
                                start=(kb == 0), stop=(kb == KBl - 1))
                        ftt = work.tile([128, FR], FT, tag="ftsb")
                        nc.scalar.copy(ftt[:, 0:F], pft[:])
                        nc.scalar.copy(ftt[:, F:F + 16], pel[:])
                        nc.sync.dma_start(ftag[l][j * 128:(j + 1) * 128, :], ftt[:])
                        if debug_dump and l == 0:
                            nc.sync.dma_start(
                                dbg["dbg_ft0"][j * 128:(j + 1) * 128, :],
                                ftt[:] if FT == bf16 else ftt[:].bitcast(f32))
                            nc.sync.dma_start(
                                dbg["dbg_eler0"][j * 128:(j + 1) * 128, :], elt[:])
                    else:
                        ft3t = work.tile([128, 64], f32, tag="ftsb")
                        nc.any.tensor_copy(ft3t[:, 0:18], pft[:, 0:18])
                        nc.any.tensor_copy(
                            res3_sb[:, j * H3:(j + 1) * H3], pft[:, 18:24])
                        nc.sync.dma_start(ft3ag[j * 128:(j + 1) * 128, :], ft3t[:])

                # AllGather the shard's ft (+eler)
                if l < 3:
                    nc.gpsimd.collective_compute(
                        "AllGather", ALU.bypass, replica_groups=rg,
                        ins=[ftag[l][:].opt()], outs=[ftg[l][:].opt()])
                else:
                    nc.gpsimd.collective_compute(
                        "AllGather", ALU.bypass, replica_groups=rg,
                        ins=[ft3ag[:].opt()], outs=[ft3g[:].opt()])

            # ================= edge phase (layers 0-2) =================
            def edge_phase(l):
                for b in range(nb_lim):
                    S_t = edge.tile([128, KB, 128], FT, tag="S")
                    nc.sync.dma_start(S_t[:], Sarr[b].rearrange("p (c d) -> p c d", d=128))
                    ixb = edge.tile([128, EB // 16], i16, tag="ixb")
                    nc.sync.dma_start(
                        ixb[:], idxX[:, b * EB // 16:(b + 1) * EB // 16])
                    X = edge.tile([128, KB, FR], FT, tag="X")
                    nc.gpsimd.dma_gather(
                        X[:], ftg[l][:], ixb[:],
                        num_idxs=EB, num_idxs_reg=EB, elem_size=FR,
                        single_packet=False)
                    ixeb = edge.tile([128, EB // 16], i16, tag="ixeb")
                    nc.sync.dma_start(
                        ixeb[:], idxE[:, b * EB // 16:(b + 1) * EB // 16])
                    EL = edge.tile([128, KB, 64], FT, tag="EL")
                    nc.gpsimd.dma_gather(
                        EL[:], ftg[l][:, F:FR], ixeb[:],
                        num_idxs=EB, num_idxs_reg=EB, elem_size=64,
                        elem_step=FR, single_packet=False)
                    et = edge.tile([128, KB, H], f32, tag="et")
                    elsrc = (X[:, 0:KB, F:F + H] if FT == bf16
                             else X[:, 0:KB, F:F + H].bitcast(f32))
                    erdst = (EL[:, 0:KB, H:2 * H] if FT == bf16
                             else EL[:, 0:KB, H:2 * H].bitcast(f32))
                    nc.vector.tensor_tensor(et[:], elsrc, erdst, ALU.add)
                    lt = edge.tile([128, KB * H], f32, tag="lt")
                    nc.vector.scalar_tensor_tensor(
                        lt[:], et[:].rearrange("p c h -> p (c h)"), NEG_SLOPE,
                        et[:].rearrange("p c h -> p (c h)"), ALU.mult, ALU.max)
                    pt = edge.tile([128, KB * H], FT, tag="pt")
                    nc.scalar.activation(pt[:], lt[:], AF.Exp)
                    Xv = X[:, :, 0:F].rearrange("p c (h d) -> p c h d", h=H)
                    Xvr = Xv if FT == bf16 else Xv.bitcast(f32)
                    ptr = pt[:] if FT == bf16 else pt[:].bitcast(f32)
                    pb = ptr.rearrange("p (c h) -> p c h", h=H) \
                        .unsqueeze(3).broadcast_to([128, KB, H, D])
                    nc.vector.tensor_tensor(Xv, Xvr, pb, ALU.mult)
                    prst = psA.tile([128, F], f32, tag="A")
                    ps = psC.tile([128, H], f32, tag="C")
                    for c in range(KB):
                        nc.tensor.matmul(
                            prst[:], S_t[:, c, :], X[:, c, 0:F],
                            start=(c == 0), stop=(c == KB - 1))
                    for c in range(KB):
                        nc.tensor.matmul(
                            ps[:], S_t[:, c, :], pt[:, c * H:(c + 1) * H],
                            start=(c == 0), stop=(c == KB - 1))
                    sse = edge.tile([128, H], f32, tag="sse")
                    nc.vector.tensor_scalar_add(sse[:], ps[:], EPS)
                    rs = edge.tile([128, H], f32, tag="rs")
                    nc.vector.reciprocal(rs[:], sse[:])
                    t1 = edge.tile([128, H, D], f32, tag="t1")
                    nc.vector.tensor_tensor(
                        t1[:], prst[:].rearrange("p (h d) -> p h d", h=H),
                        rs[:].unsqueeze(2).broadcast_to([128, H, D]), ALU.mult)
                    t1f = t1[:].rearrange("p h d -> p (h d)")
                    t2 = edge.tile([128, F], f32, tag="t2")
                    if l == 0:
                        nc.vector.tensor_tensor(
                            t2[:], t1f, bias_sb[:, l * F:(l + 1) * F], ALU.add)
                    else:
                        hres = edge.tile([128, F], f32, tag="hres")
                        nc.sync.dma_start(
                            hres[:], h_in[l][b * 128:(b + 1) * 128, :])
                        t2a = edge.tile([128, F], f32, tag="t2a")
                        nc.gpsimd.tensor_tensor(t2a[:], t1f, hres[:], ALU.add)
                        nc.vector.tensor_tensor(
                            t2[:], t2a[:], bias_sb[:, l * F:(l + 1) * F], ALU.add)
                    # ELU
                    mm = edge.tile([128, F], f32, tag="mm")
                    nc.vector.tensor_scalar_min(mm[:], t2[:], 0.0)
                    ex = edge.tile([128, F], f32, tag="ex")
                    nc.scalar.activation(ex[:], mm[:], AF.Exp)
                    rl = edge.tile([128, F], f32, tag="rl")
                    nc.scalar.activation(rl[:], t2[:], AF.Relu)
                    hn = edge.tile([128, F], f32, tag="hn")
                    nc.vector.scalar_tensor_tensor(
                        hn[:], ex[:], -1.0, rl[:], ALU.add, ALU.add)
                    nc.sync.dma_start(
                        h_in[l + 1][b * 128:(b + 1) * 128, :], hn[:])
                    if debug_dump:
                        nc.sync.dma_start(
                            dbg[f"dbg_h{l + 1}"][b * 128:(b + 1) * 128, :], hn[:])

            # ================= edge phase (layer 3) + pooling ===============
            def edge_phase3():
                ppool = psP.tile([G, H3], f32, tag="P")
                for b in range(NB):
                    S_t = edge.tile([128, KB, 128], FT, tag="S")
                    nc.sync.dma_start(S_t[:], Sarr[b].rearrange("p (c d) -> p c d", d=128))
                    ixb3 = edge.tile([128, EB // 16], i16, tag="ixb")
                    nc.sync.dma_start(
                        ixb3[:], idxX[:, b * EB // 16:(b + 1) * EB // 16])
                    XS = edge.tile([128, KB, 64], f32, tag="X")
                    nc.gpsimd.dma_gather(
                        XS[:], ft3g[:], ixb3[:],
                        num_idxs=EB, num_idxs_reg=EB, elem_size=64,
                        single_packet=False)
                    ixeb = edge.tile([128, EB // 16], i16, tag="ixeb")
                    nc.sync.dma_start(
                        ixeb[:], idxE[:, b * EB // 16:(b + 1) * EB // 16])
                    EL = edge.tile([128, KB, 64], f32, tag="EL")
                    nc.gpsimd.dma_gather(
                        EL[:], ft3g[:], ixeb[:],
                        num_idxs=EB, num_idxs_reg=EB, elem_size=64,
                        single_packet=False)
                    et = edge.tile([128, KB, H3], f32, tag="et")
                    nc.vector.tensor_tensor(
                        et[:], XS[:, 0:KB, 6:12], EL[:, 0:KB, 12:18], ALU.add)
                    lt = edge.tile([128, KB * H3], f32, tag="lt")
                    nc.vector.scalar_tensor_tensor(
                        lt[:], et[:].rearrange("p c h -> p (c h)"), NEG_SLOPE,
                        et[:].rearrange("p c h -> p (c h)"), ALU.mult, ALU.max)
                    XP = edge.tile([128, KB, 2 * H3], FT, tag="pt")
                    nc.scalar.activation(
                        XP[:, :, H3:2 * H3],
                        lt[:].rearrange("p (c h) -> p c h", h=H3), AF.Exp)
                    xpr = (XP[:, :, H3:2 * H3] if FT == bf16
                           else XP[:, :, H3:2 * H3].bitcast(f32))
                    nc.vector.tensor_tensor(
                        XP[:, :, 0:H3], XS[:, 0:KB, 0:H3], xpr, ALU.mult)
                    prst = psC.tile([128, 2 * H3], f32, tag="C")
                    for c in range(KB):
                        nc.tensor.matmul(
                            prst[:], S_t[:, c, :], XP[:, c, :],
                            start=(c == 0), stop=(c == KB - 1))
                    sse = edge.tile([128, H3], f32, tag="sse")
                    nc.vector.tensor_scalar_add(sse[:], prst[:, H3:2 * H3], EPS)
                    rs = edge.tile([128, H3], f32, tag="rs")
                    nc.vector.reciprocal(rs[:], sse[:])
                    t1 = edge.tile([128, H3], f32, tag="t1")
                    nc.vector.tensor_tensor(t1[:], prst[:, 0:H3], rs[:], ALU.mult)
                    h3 = edge.tile([128, H3], f32r, tag="hn3")
                    nc.vector.tensor_tensor(
                        h3[:], t1[:], res3_sb[:, b * H3:(b + 1) * H3], ALU.add)
                    if debug_dump:
                        nc.sync.dma_start(
                            dbg["dbg_hfin"][b * 128:(b + 1) * 128, :],
                            h3[:].bitcast(f32))
                    if l3_lim >= 2:
                        Gt = edge.tile([128, G], f32r, tag="Gt")
                        nc.sync.dma_start(Gt[:], Gh[b])
                        nc.tensor.matmul(
                            ppool[:], Gt[:], h3[:],
                            start=(b == 0), stop=(b == NB - 1))
                if l3_lim < 3:
                    return
                # readout
                pol = work.tile([G, H3], f32, tag="pol")
                nc.any.tensor_copy(pol[:], ppool[:])
                nc.sync.dma_start(ar_in[:], pol[:])
                if debug_dump:
                    nc.sync.dma_start(dbg["dbg_pol"][:], pol[:])
                nc.gpsimd.collective_compute(
                    "AllReduce", ALU.add, replica_groups=rg,
                    ins=[ar_in[:].opt()], outs=[ar_out[:].opt()])
                pol2 = work.tile([G, H3], f32, tag="pol2")
                nc.sync.dma_start(pol2[:], ar_out[:])
                if debug_dump:
                    nc.sync.dma_start(dbg["dbg_pol2"][:], pol2[:])
                pr = work.tile([G, H3], f32, tag="pr")
                nc.vector.tensor_tensor(pr[:], pol2[:], lin_sb[0:G, 0:H3], ALU.mult)
                ro = work.tile([G, 1], f32, tag="ro")
                nc.vector.tensor_reduce(ro[:], pr[:], axis=AX.X, op=ALU.add)
                ro2 = work.tile([G, 1], f32, tag="ro2")
                nc.vector.tensor_tensor(
                    ro2[:], ro[:], lin_sb[0:G, H3:H3 + 1], ALU.add)
                nc.sync.dma_start(out[:], ro2[:])

            steps = [lambda: dense_phase(0), lambda: edge_phase(0),
                     lambda: dense_phase(1), lambda: edge_phase(1),
                     lambda: dense_phase(2), lambda: edge_phase(2),
                     lambda: dense_phase(3), edge_phase3]
            for st in steps[:phases]:
                st()

    nc.compile()
    return nc


def kernel(**inputs):
    in_maps, KB = preprocess(inputs)
    ft_name = "float32" if FT_DT == f32 else "bfloat16"
    nc = build_program(KB, ft_name, LAST.get("debug_dump", False),
                       LAST.get("phases", 8), LAST.get("nb_lim", NB),
                       LAST.get("l3_lim", 3))
    br = run_bass_kernel_spmd(
        nc, in_maps, core_ids=list(range(NCORES)), trace=TRACE, **TRACE_KW)
    LAST["br"] = br
    return np.asarray(br.results[0]["out"], dtype=np.float32)



# revision 10
# speedup vs baseline: 6.3185x; 6.3185x over previous
"""GAT (4-layer, 8-head) Trainium2 kernel, 8-core SPMD.

Strategy (per sharding hint): nodes partitioned into 8 contiguous shards
(4096 nodes/core); each core owns the edges whose dst falls in its shard
(segment softmax + scatter stay local). Per-layer, each core computes the
dense ft = h @ W for its shard, then an AllGather replicates ft (+ the
attention logit component el) so every core can gather remote src rows
with dma_gather. er[dst] is gathered from a small core-local table (dst is
always local). Edge aggregation (segment softmax + weighted scatter-add)
is done as PE matmuls against host-precomputed one-hot dst matrices, with
the per-edge exp() folded into the moving operand. Weights are replicated.

The gathered ft table is stored bf16 (row = 512 bf16 ft | 8 f32 el | 8 f32
er carried as raw bits | pad to 640); logits stay f32 end-to-end, only the
aggregated features and exp weights are bf16.

Softmax: the reference subtracts a per-segment max; mathematically alpha is
invariant to any constant shift, and the logit range here is [-7, 7], so we
exp() directly (f32 safe) and normalize by the segment sum at the node side.
"""

import functools

import numpy as np

import concourse.bacc as bacc
import concourse.bass as bass
import concourse.mybir as mybir
import concourse.tile as tile
from concourse.bass_utils import run_bass_kernel_spmd

# ---- problem constants (hardcoded per contract) ----
N, E, G = 32768, 262144, 64
NCORES = 8
SH = N // NCORES          # 4096 nodes per core
NB = SH // 128            # 32 dst blocks per core
NCH = NB                  # node chunks per core (same 128-partition blocks)
F0, F = 128, 512
H, D = 8, 64
H3 = 6
NEG_SLOPE = 0.2
EPS = 1e-30

f32 = mybir.dt.float32
f32r = mybir.dt.float32r
bf16 = mybir.dt.bfloat16
i16 = mybir.dt.int16

# storage dtype of the gathered ft / one-hot S / edge moving operand
FT_DT = bf16

TRACE = False
TRACE_KW = {}
LAST = {}

AF = mybir.ActivationFunctionType
ALU = mybir.AluOpType
AX = mybir.AxisListType


def _wrap_idx(v):
    """int16 gather-index layout: element i at [i%16, i//16], replicated to
    128 partitions."""
    L = len(v)
    w = np.zeros((16, L // 16), np.int16)
    w[np.arange(L) % 16, np.arange(L) // 16] = v.astype(np.int16)
    return np.tile(w, (8, 1))


def preprocess(inputs):
    src = np.asarray(inputs["src"]).astype(np.int64)
    dst = np.asarray(inputs["dst"]).astype(np.int64)
    graph_id = np.asarray(inputs["graph_id"]).astype(np.int64)
    feat = np.asarray(inputs["feat"], dtype=np.float32)

    per_core_edges = []
    KB = 0
    for c in range(NCORES):
        m = (dst >= c * SH) & (dst < (c + 1) * SH)
        es, ed = src[m], dst[m]
        o = np.argsort(ed, kind="stable")
        es, ed = es[o], ed[o]
        dl = ed - c * SH
        blk = dl >> 7
        counts = np.bincount(blk, minlength=NB)
        KB = max(KB, int(np.ceil(counts.max() / 128)))
        per_core_edges.append((es, ed, dl, blk, counts))
    EB = KB * 128

    if FT_DT == bf16:
        try:
            ft_np = np.dtype("bfloat16")
            np.zeros(1, ft_np)
        except TypeError:
            import ml_dtypes
            ft_np = ml_dtypes.bfloat16
    else:
        ft_np = np.float32

    # shared weight-derived arrays
    def Amat(al):  # [1,H,D] -> [H*D, H]
        al = np.asarray(al, np.float64)[0]
        hh, dd = al.shape
        A = np.zeros((hh * dd, hh), np.float64)
        for h in range(hh):
            A[h * dd:(h + 1) * dd, h] = al[h]
        return A

    W0 = np.asarray(inputs["W0"], np.float64)
    W1 = np.asarray(inputs["W1"], np.float64)
    W2 = np.asarray(inputs["W2"], np.float64)
    W3 = np.asarray(inputs["W3"], np.float64)
    resW3 = np.asarray(inputs["resW3"], np.float64)
    al3 = np.asarray(inputs["al3"], np.float64)[0, :, 0]
    ar3 = np.asarray(inputs["ar3"], np.float64)[0, :, 0]

    WA = {}
    for l, W in ((0, W0), (1, W1), (2, W2)):
        Aal = Amat(inputs[f"al{l}"])
        Aar = Amat(inputs[f"ar{l}"])
        WA[l] = np.concatenate([W @ Aal, W @ Aar], axis=1).astype(np.float32)
    W3c = np.concatenate(
        [W3, W3 * al3[None, :], W3 * ar3[None, :], resW3], axis=1
    ).astype(np.float32)  # [512, 24]
    b3row = np.zeros((1, 24), np.float32)
    b3row[0, 18:24] = np.asarray(inputs["b3"], np.float32)

    bias_bc = np.tile(
        np.concatenate([np.asarray(inputs[f"b{l}"], np.float32)
                        for l in range(3)])[None, :], (128, 1))  # [128, 3*512]
    lin_bc = np.zeros((128, H3 + 1), np.float32)
    lin_bc[:, 0:H3] = np.asarray(inputs["linW"], np.float32)[:, 0][None, :]
    lin_bc[:, H3] = float(np.asarray(inputs["linb"], np.float32)[0])
    ident = np.eye(128, dtype=np.float32)

    shared = {
        "W0": W0.astype(np.float32),
        "W1": W1.astype(np.float32),
        "W2": W2.astype(np.float32),
        "W3c": W3c,
        "WA0": WA[0], "WA1": WA[1], "WA2": WA[2],
        "b3row": b3row,
        "bias_bc": bias_bc,
        "lin_bc": lin_bc,
        "identity": ident,
        "ones1": np.ones((1, 128), np.float32),
    }

    in_maps = []
    eye64 = np.eye(G, dtype=np.float32)
    for c in range(NCORES):
        es, ed, dl, blk, counts = per_core_edges[c]
        offs = np.concatenate([[0], np.cumsum(counts)])
        idxX = np.zeros((128, NB * EB // 16), np.int16)
        idxE = np.zeros((128, NB * EB // 16), np.int16)
        Sarr = np.zeros((NB, 128, EB), np.float32)
        for b in range(NB):
            s_b = es[offs[b]:offs[b + 1]]
            dloc = dl[offs[b]:offs[b + 1]] - b * 128
            dl_b = dl[offs[b]:offs[b + 1]]
            npad = EB - len(s_b)
            s_pad = np.concatenate([s_b, np.zeros(npad, np.int64)])
            dg_pad = np.concatenate([dl_b, np.zeros(npad, np.int64)])
            idxX[:, b * EB // 16:(b + 1) * EB // 16] = _wrap_idx(s_pad)
            idxE[:, b * EB // 16:(b + 1) * EB // 16] = _wrap_idx(dg_pad)
            j = np.arange(len(dloc))
            S3 = Sarr[b].reshape(128, KB, 128)
            S3[j % 128, j // 128, dloc] = 1.0
        gid = graph_id[c * SH:(c + 1) * SH]
        Gh = eye64[gid].reshape(NB, 128, G)
        im = dict(shared)
        im["feat_sh"] = feat[c * SH:(c + 1) * SH]
        im["idxX"] = idxX
        im["idxE"] = idxE
        im["Sarr"] = Sarr.astype(ft_np)
        im["Gh"] = Gh
        in_maps.append(im)
    return in_maps, KB


@functools.lru_cache(maxsize=16)
def build_program(KB, ft_dt_name, debug_dump=False, phases=8, nb_lim=NB,
                  l3_lim=3, reps=1, ag_lim=4, shared_ag=True, edge_bufs=3):
    FT = {"float32": f32, "bfloat16": bf16}[ft_dt_name]
    FTB = mybir.dt.size(FT)      # bytes per stored ft element
    EB = KB * 128
    # packed gathered row: ft | el(8 f32) er(8 f32) as raw bits | pad.
    # row stride bytes must be a multiple of 256 for dma_gather.
    FR = 640 if FT == bf16 else 576
    ELW = 32 if FT == bf16 else 16   # el/er width in FT elements (16 f32)
    nc = bacc.Bacc("TRN2", target_bir_lowering=False, debug=False)

    # ---- I/O ----
    feat_sh = nc.dram_tensor("feat_sh", [SH, F0], f32, kind="ExternalInput")
    Wt = {
        0: nc.dram_tensor("W0", [F0, F], f32r, kind="ExternalInput"),
        1: nc.dram_tensor("W1", [F, F], f32r, kind="ExternalInput"),
        2: nc.dram_tensor("W2", [F, F], f32r, kind="ExternalInput"),
        3: nc.dram_tensor("W3c", [F, 24], f32r, kind="ExternalInput"),
    }
    WAt = {l: nc.dram_tensor(f"WA{l}", [F0 if l == 0 else F, 16], f32r,
                             kind="ExternalInput") for l in range(3)}
    b3row = nc.dram_tensor("b3row", [1, 24], f32r, kind="ExternalInput")
    bias_bc = nc.dram_tensor("bias_bc", [128, 3 * F], f32, kind="ExternalInput")
    lin_bc = nc.dram_tensor("lin_bc", [128, H3 + 1], f32, kind="ExternalInput")
    identity = nc.dram_tensor("identity", [128, 128], f32, kind="ExternalInput")
    ones1 = nc.dram_tensor("ones1", [1, 128], f32r, kind="ExternalInput")
    idxX = nc.dram_tensor("idxX", [128, NB * EB // 16], i16, kind="ExternalInput")
    idxE = nc.dram_tensor("idxE", [128, NB * EB // 16], i16, kind="ExternalInput")
    Sarr = nc.dram_tensor("Sarr", [NB, 128, EB], FT, kind="ExternalInput")
    Gh = nc.dram_tensor("Gh", [NB, 128, G], f32r, kind="ExternalInput")
    out = nc.dram_tensor("out", [G, 1], f32, kind="ExternalOutput")
    dbg = {}
    if debug_dump:
        for nm, shp in (("dbg_h1", [SH, F]), ("dbg_h2", [SH, F]),
                        ("dbg_h3", [SH, F]), ("dbg_eler0", [SH, 16]),
                        ("dbg_hfin", [SH, H3]),
                        ("dbg_pol", [G, H3]), ("dbg_pol2", [G, H3])):
            dbg[nm] = nc.dram_tensor(nm, shp, f32, kind="ExternalOutput")

    rg = [list(range(NCORES))]
    ag_space = "Shared" if shared_ag else "Local"

    with tile.TileContext(nc) as tc:
        with (
            tc.tile_pool(name="const", bufs=1) as constp,
            tc.tile_pool(name="wpool", bufs=2) as wpool,
            tc.tile_pool(name="work", bufs=3) as work,
            tc.tile_pool(name="edge", bufs=edge_bufs) as edge,
            tc.tile_pool(name="psA", bufs=2, space="PSUM") as psA,
            tc.tile_pool(name="psB", bufs=2, space="PSUM") as psB,
            tc.tile_pool(name="psC", bufs=2, space="PSUM") as psC,
            tc.tile_pool(name="psP", bufs=1, space="PSUM") as psP,
            tc.tile_pool(name="dram", bufs=1, space="DRAM") as dram,
        ):
            # ---- resident constants ----
            ident_sb = constp.tile([128, 128], f32)
            nc.sync.dma_start(ident_sb[:], identity[:])
            ones_sb = constp.tile([1, 128], f32r)
            nc.sync.dma_start(ones_sb[:], ones1[:])
            b3r_sb = constp.tile([1, 24], f32r)
            nc.sync.dma_start(b3r_sb[:], b3row[:])
            lin_sb = constp.tile([128, H3 + 1], f32)
            nc.sync.dma_start(lin_sb[:], lin_bc[:])
            bias_sb = constp.tile([128, 3 * F], f32)
            nc.sync.dma_start(bias_sb[:], bias_bc[:])
            res3_sb = constp.tile([128, NCH * H3], f32)

            # ---- internal DRAM arrays ----
            h_in = {l: dram.tile([SH, F], f32, name=f"h{l}") for l in (1, 2, 3)}
            ftag = {l: dram.tile([SH, FR], FT, name=f"ftag{l}") for l in range(3)}
            ftg = {l: dram.tile([N, FR], FT, name=f"ftg{l}",
                               addr_space=ag_space) for l in range(3)}
            eler = {l: dram.tile([SH, 64], f32, name=f"eler{l}")
                    for l in range(3)}
            ft3ag = dram.tile([SH, 64], f32, name="ft3ag")
            ft3g = dram.tile([N, 64], f32, name="ft3g", addr_space=ag_space)
            ar_in = dram.tile([G, H3], f32, name="arin")
            ar_out = dram.tile([G, H3], f32, name="arout")

            # ================= dense phase =================
            def dense_phase(l):
                K = F0 if l == 0 else F
                KBl = K // 128
                # layer weights -> SBUF
                w_sb = wpool.tile([128, KBl, F if l < 3 else 24], f32r, tag="W")
                nc.sync.dma_start(
                    w_sb[:], Wt[l][:].rearrange("(kb p) f -> p kb f", p=128))
                if l < 3:
                    wa_sb = wpool.tile([128, KBl, 16], f32r, tag="WA")
                    nc.sync.dma_start(
                        wa_sb[:], WAt[l][:].rearrange("(kb p) f -> p kb f", p=128))
                h_src = feat_sh if l == 0 else h_in[l]
                for j in range(NCH):
                    hl = work.tile([128, K], f32, tag="hload")
                    nc.sync.dma_start(hl[:], h_src[j * 128:(j + 1) * 128, :])
                    pT = psA.tile([128, K], f32, tag="A")
                    for kb in range(KBl):
                        nc.tensor.transpose(
                            pT[:, kb * 128:(kb + 1) * 128],
                            hl[:, kb * 128:(kb + 1) * 128], ident_sb[:])
                    hT = work.tile([128, K], f32r, tag="hT")
                    nc.scalar.copy(hT[:], pT[:])
                    FW = F if l < 3 else 24
                    pft = psB.tile([128, FW], f32, tag="B")
                    for kb in range(KBl):
                        nc.tensor.matmul(
                            pft[:], hT[:, kb * 128:(kb + 1) * 128],
                            w_sb[:, kb, :],
                            start=(kb == 0), stop=(kb == KBl - 1 and l < 3))
                    if l == 3:
                        nc.tensor.matmul(pft[:], ones_sb[:], b3r_sb[:],
                                         start=False, stop=True)
                    if l < 3:
                        pel = psC.tile([128, 16], f32, tag="C")
                        for kb in range(KBl):
                            nc.tensor.matmul(
                                pel[:], hT[:, kb * 128:(kb + 1) * 128],
                                wa_sb[:, kb, :],
                                start=(kb == 0), stop=(kb == KBl - 1))
                        ftt = work.tile([128, FR], FT, tag="ftsb")
                        nc.scalar.copy(ftt[:, 0:F], pft[:])
                        # el/er carried as raw f32 bits inside the FT row
                        nc.scalar.copy(
                            ftt[:, F:F + ELW].bitcast(f32), pel[:])
                        nc.sync.dma_start(ftag[l][j * 128:(j + 1) * 128, :], ftt[:])
                        # local dst-side table: el|er f32 at cols 0:16
                        nc.sync.dma_start(
                            eler[l][j * 128:(j + 1) * 128, 0:16],
                            ftt[:, F:F + ELW].bitcast(f32))
                        if debug_dump and l == 0:
                            nc.sync.dma_start(
                                dbg["dbg_eler0"][j * 128:(j + 1) * 128, :],
                                ftt[:, F:F + ELW].bitcast(f32))
                    else:
                        ft3t = work.tile([128, 64], f32, tag="ftsb")
                        nc.any.tensor_copy(ft3t[:, 0:18], pft[:, 0:18])
                        nc.any.tensor_copy(
                            res3_sb[:, j * H3:(j + 1) * H3], pft[:, 18:24])
                        nc.sync.dma_start(ft3ag[j * 128:(j + 1) * 128, :], ft3t[:])

                # AllGather the shard's ft (+el)
                if l >= ag_lim:
                    return
                if l < 3:
                    nc.gpsimd.collective_compute(
                        "AllGather", ALU.bypass, replica_groups=rg,
                        ins=[ftag[l][:].opt()], outs=[ftg[l][:].opt()])
                else:
                    nc.gpsimd.collective_compute(
                        "AllGather", ALU.bypass, replica_groups=rg,
                        ins=[ft3ag[:].opt()], outs=[ft3g[:].opt()])

            # ================= edge phase (layers 0-2) =================
            def edge_phase(l):
                for b in range(nb_lim):
                    S_t = edge.tile([128, KB, 128], FT, tag="S")
                    nc.sync.dma_start(S_t[:], Sarr[b].rearrange("p (c d) -> p c d", d=128))
                    ixb = edge.tile([128, EB // 16], i16, tag="ixb")
                    nc.sync.dma_start(
                        ixb[:], idxX[:, b * EB // 16:(b + 1) * EB // 16])
                    X = edge.tile([128, KB, FR], FT, tag="X")
                    nc.gpsimd.dma_gather(
                        X[:], ftg[l][:], ixb[:],
                        num_idxs=EB, num_idxs_reg=EB, elem_size=FR,
                        single_packet=False)
                    ixeb = edge.tile([128, EB // 16], i16, tag="ixeb")
                    nc.sync.dma_start(
                        ixeb[:], idxE[:, b * EB // 16:(b + 1) * EB // 16])
                    EL = edge.tile([128, KB, 64], f32, tag="EL")
                    nc.gpsimd.dma_gather(
                        EL[:], eler[l][:], ixeb[:],
                        num_idxs=EB, num_idxs_reg=EB, elem_size=64,
                        single_packet=False)
                    et = edge.tile([128, KB, H], f32, tag="et")
                    elsrc = (X[:, 0:KB, F:F + 16] if FT == bf16
                             else X[:, 0:KB, F:F + H]).bitcast(f32)
                    erdst = EL[:, 0:KB, 8:16]
                    nc.vector.tensor_tensor(et[:], elsrc, erdst, ALU.add)
                    lt = edge.tile([128, KB * H], f32, tag="lt")
                    nc.vector.scalar_tensor_tensor(
                        lt[:], et[:].rearrange("p c h -> p (c h)"), NEG_SLOPE,
                        et[:].rearrange("p c h -> p (c h)"), ALU.mult, ALU.max)
                    pt = edge.tile([128, KB * H], FT, tag="pt")
                    nc.scalar.activation(pt[:], lt[:], AF.Exp)
                    Xv = X[:, :, 0:F].rearrange("p c (h d) -> p c h d", h=H)
                    Xvr = Xv if FT == bf16 else Xv.bitcast(f32)
                    ptr = pt[:] if FT == bf16 else pt[:].bitcast(f32)
                    pb = ptr.rearrange("p (c h) -> p c h", h=H) \
                        .unsqueeze(3).broadcast_to([128, KB, H, D])
                    nc.vector.tensor_tensor(Xvr, Xvr, pb, ALU.mult)
                    prst = psA.tile([128, F], f32, tag="A")
                    ps = psC.tile([128, H], f32, tag="C")
                    for c in range(KB):
                        nc.tensor.matmul(
                            prst[:], S_t[:, c, :], X[:, c, 0:F],
                            start=(c == 0), stop=(c == KB - 1))
                    for c in range(KB):
                        nc.tensor.matmul(
                            ps[:], S_t[:, c, :], pt[:, c * H:(c + 1) * H],
                            start=(c == 0), stop=(c == KB - 1))
                    sse = edge.tile([128, H], f32, tag="sse")
                    nc.vector.tensor_scalar_add(sse[:], ps[:], EPS)
                    rs = edge.tile([128, H], f32, tag="rs")
                    nc.vector.reciprocal(rs[:], sse[:])
                    t1 = edge.tile([128, H, D], f32, tag="t1")
                    nc.vector.tensor_tensor(
                        t1[:], prst[:].rearrange("p (h d) -> p h d", h=H),
                        rs[:].unsqueeze(2).broadcast_to([128, H, D]), ALU.mult)
                    t1f = t1[:].rearrange("p h d -> p (h d)")
                    t2 = edge.tile([128, F], f32, tag="t2")
                    if l == 0:
                        nc.vector.tensor_tensor(
                            t2[:], t1f, bias_sb[:, l * F:(l + 1) * F], ALU.add)
                    else:
                        hres = edge.tile([128, F], f32, tag="hres")
                        nc.sync.dma_start(
                            hres[:], h_in[l][b * 128:(b + 1) * 128, :])
                        t2a = edge.tile([128, F], f32, tag="t2a")
                        nc.vector.tensor_tensor(t2a[:], t1f, hres[:], ALU.add)
                        nc.vector.tensor_tensor(
                            t2[:], t2a[:], bias_sb[:, l * F:(l + 1) * F], ALU.add)
                    # ELU
                    mm = edge.tile([128, F], f32, tag="mm")
                    nc.vector.tensor_scalar_min(mm[:], t2[:], 0.0)
                    ex = edge.tile([128, F], f32, tag="ex")
                    nc.scalar.activation(ex[:], mm[:], AF.Exp)
                    rl = edge.tile([128, F], f32, tag="rl")
                    nc.scalar.activation(rl[:], t2[:], AF.Relu)
                    hn = edge.tile([128, F], f32, tag="hn")
                    nc.vector.scalar_tensor_tensor(
                        hn[:], ex[:], -1.0, rl[:], ALU.add, ALU.add)
                    nc.sync.dma_start(
                        h_in[l + 1][b * 128:(b + 1) * 128, :], hn[:])
                    if debug_dump:
                        nc.sync.dma_start(
                            dbg[f"dbg_h{l + 1}"][b * 128:(b + 1) * 128, :], hn[:])

            # ================= edge phase (layer 3) + pooling ===============
            def edge_phase3():
                ppool = psP.tile([G, H3], f32, tag="P")
                for b in range(NB):
                    S_t = edge.tile([128, KB, 128], FT, tag="S")
                    nc.sync.dma_start(S_t[:], Sarr[b].rearrange("p (c d) -> p c d", d=128))
                    ixb3 = edge.tile([128, EB // 16], i16, tag="ixb")
                    nc.sync.dma_start(
                        ixb3[:], idxX[:, b * EB // 16:(b + 1) * EB // 16])
                    XS = edge.tile([128, KB, 64], f32, tag="X3")
                    nc.gpsimd.dma_gather(
                        XS[:], ft3g[:], ixb3[:],
                        num_idxs=EB, num_idxs_reg=EB, elem_size=64,
                        single_packet=False)
                    ixeb = edge.tile([128, EB // 16], i16, tag="ixeb")
                    nc.sync.dma_start(
                        ixeb[:], idxE[:, b * EB // 16:(b + 1) * EB // 16])
                    EL = edge.tile([128, KB, 64], f32, tag="EL")
                    nc.gpsimd.dma_gather(
                        EL[:], ft3ag[:], ixeb[:],
                        num_idxs=EB, num_idxs_reg=EB, elem_size=64,
                        single_packet=False)
                    et = edge.tile([128, KB, H3], f32, tag="et")
                    nc.vector.tensor_tensor(
                        et[:], XS[:, 0:KB, 6:12], EL[:, 0:KB, 12:18], ALU.add)
                    lt = edge.tile([128, KB * H3], f32, tag="lt")
                    nc.vector.scalar_tensor_tensor(
                        lt[:], et[:].rearrange("p c h -> p (c h)"), NEG_SLOPE,
                        et[:].rearrange("p c h -> p (c h)"), ALU.mult, ALU.max)
                    XP = edge.tile([128, KB, 2 * H3], FT, tag="pt3")
                    nc.scalar.activation(
                        XP[:, :, H3:2 * H3],
                        lt[:].rearrange("p (c h) -> p c h", h=H3), AF.Exp)
                    xpr = (XP[:, :, H3:2 * H3] if FT == bf16
                           else XP[:, :, H3:2 * H3].bitcast(f32))
                    xpo = (XP[:, :, 0:H3] if FT == bf16
                           else XP[:, :, 0:H3].bitcast(f32))
                    nc.vector.tensor_tensor(
                        xpo, XS[:, 0:KB, 0:H3], xpr, ALU.mult)
                    prst = psC.tile([128, 2 * H3], f32, tag="C")
                    for c in range(KB):
                        nc.tensor.matmul(
                            prst[:], S_t[:, c, :], XP[:, c, :],
                            start=(c == 0), stop=(c == KB - 1))
                    sse = edge.tile([128, H3], f32, tag="sse")
                    nc.vector.tensor_scalar_add(sse[:], prst[:, H3:2 * H3], EPS)
                    rs = edge.tile([128, H3], f32, tag="rs")
                    nc.vector.reciprocal(rs[:], sse[:])
                    t1 = edge.tile([128, H3], f32, tag="t13")
                    nc.vector.tensor_tensor(t1[:], prst[:, 0:H3], rs[:], ALU.mult)
                    h3 = edge.tile([128, H3], f32r, tag="hn3")
                    nc.vector.tensor_tensor(
                        h3[:], t1[:], res3_sb[:, b * H3:(b + 1) * H3], ALU.add)
                    if debug_dump:
                        nc.sync.dma_start(
                            dbg["dbg_hfin"][b * 128:(b + 1) * 128, :],
                            h3[:].bitcast(f32))
                    if l3_lim >= 2:
                        Gt = edge.tile([128, G], f32r, tag="Gt")
                        nc.sync.dma_start(Gt[:], Gh[b])
                        nc.tensor.matmul(
                            ppool[:], Gt[:], h3[:],
                            start=(b == 0), stop=(b == NB - 1))
                if l3_lim < 3:
                    return
                # readout
                pol = work.tile([G, H3], f32, tag="pol")
                nc.any.tensor_copy(pol[:], ppool[:])
                nc.sync.dma_start(ar_in[:], pol[:])
                if debug_dump:
                    nc.sync.dma_start(dbg["dbg_pol"][:], pol[:])
                nc.gpsimd.collective_compute(
                    "AllReduce", ALU.add, replica_groups=rg,
                    ins=[ar_in[:].opt()], outs=[ar_out[:].opt()])
                pol2 = work.tile([G, H3], f32, tag="pol2")
                nc.sync.dma_start(pol2[:], ar_out[:])
                if debug_dump:
                    nc.sync.dma_start(dbg["dbg_pol2"][:], pol2[:])
                pr = work.tile([G, H3], f32, tag="pr")
                nc.vector.tensor_tensor(pr[:], pol2[:], lin_sb[0:G, 0:H3], ALU.mult)
                ro = work.tile([G, 1], f32, tag="ro")
                nc.vector.tensor_reduce(ro[:], pr[:], axis=AX.X, op=ALU.add)
                ro2 = work.tile([G, 1], f32, tag="ro2")
                nc.vector.tensor_tensor(
                    ro2[:], ro[:], lin_sb[0:G, H3:H3 + 1], ALU.add)
                nc.sync.dma_start(out[:], ro2[:])

            steps = [lambda: dense_phase(0), lambda: edge_phase(0),
                     lambda: dense_phase(1), lambda: edge_phase(1),
                     lambda: dense_phase(2), lambda: edge_phase(2),
                     lambda: dense_phase(3), edge_phase3]
            for _ in range(reps):
                for st in steps[:phases]:
                    st()

    nc.compile()
    return nc


def kernel(**inputs):
    in_maps, KB = preprocess(inputs)
    ft_name = "float32" if FT_DT != bf16 else "bfloat16"
    nc = build_program(KB, ft_name, LAST.get("debug_dump", False),
                       LAST.get("phases", 8), LAST.get("nb_lim", NB),
                       LAST.get("l3_lim", 3), LAST.get("reps", 1),
                       LAST.get("ag_lim", 4), LAST.get("shared_ag", True),
                       LAST.get("edge_bufs", 3))
    br = run_bass_kernel_spmd(
        nc, in_maps, core_ids=list(range(NCORES)), trace=TRACE, **TRACE_KW)
    LAST["br"] = br
    return np.asarray(br.results[0]["out"], dtype=np.float32)
